# revision 1
# baseline (speedup 1.0000x reference)
"""CapsuleLayer (dynamic routing) Trainium2 Bass kernel.

Full-input contract: kernel(inputs, W) -> [256, 10, 16, 1] f32.
Data-parallel over batch: 8 cores x 32 batches, W replicated.

Math restructuring vs the reference:
  - routing logits are always b_t = u_hat * V_t with V_t = sum of previous
    squashed outputs (broadcast over IC), so no [B,NC,IC,DC] logits tensor is
    ever materialized; only the running V[b,n,d] is kept.
  - pass 1 (uniform softmax) reduces to s1 = 0.1 * sum_i u_hat, accumulated
    on the DVE during u_hat production (idle there) + one PE partition-fold.

Everything is kept in f32: the routing iteration chaotically amplifies
perturbations (~700x), so bf16/f16 intermediates destroy accuracy. The only
reduced-precision concession is float32r (~12-bit mantissa) on the PASS-3
capsule-fold matmuls, whose rounding hits the output directly without
amplification; pass-2 folds stay plain f32 (their error feeds back through V
and grows ~20-40x). Measured: 4.8e-4 absmax vs the f32 reference.

Per core the 32 local batches are processed as 2 serial sub-batches of 16 so
the f32 u_hat stays SBUF-resident (no DRAM spill):
  partition p = i8*16 + b   (8 input capsules packed per "group", 144 groups)
  u_hat: 12 chunk tiles [128, 12*160] in a 12-slot pool; sub-batch 2's
    production reuses sub-batch 1's slots chunk-by-chunk, so its DMA/PE work
    overlaps sub-batch 1's final routing pass.
  production: u[p, g*160+nd] = sum_{r=(i8,k)} LT[r, g*128+p] * WR[r, g*160+nd]
    with LT the block-diagonal input transposes (rows r = i8*8+k, 16-col
    diagonal blocks) and WR the matching W slices - both prepacked on host.
  capsule fold: s[b, nd] = sum_p mask[p, b] * w[p, nd] via PE (mask = eye(16)
    tiled 8x), accumulated 3 groups wide ([16, 480] PSUM) to amortize weight
    loads; the 3 column blocks are summed on DVE afterwards.
Engine split per routing pass chunk (all elementwise on DVE - measured
faster on HW than offloading to GpSimd, whose software loops handle the
strided/broadcast access patterns poorly):
  DVE: logits-mult, softmax-denominator reduce, reciprocal, normalize-mult,
  weight-mult; ScalarE: exp (+ PSUM->SBUF copies); PE: matmuls/capsule folds.
"""

import os
import sys

import numpy as np

sys.path.insert(0, "/opt/trn_rl_repo")

B, IC, ID = 256, 1152, 8
NC, DC = 10, 16
NCORES = 8
BC = B // NCORES            # 32 batches per core
SB = 2                      # sub-batches per core
BB = BC // SB               # 16 batches per sub-batch
IPK = 8                     # input capsules packed per group
G2 = IC // IPK              # 144 groups
K2 = IPK * ID               # 64 contraction rows
ND = NC * DC                # 160
FREE2 = G2 * ND             # 23040
CH = 12                     # chunk size in groups (production & routing)
NCH = G2 // CH              # 12 chunks
PSUM_GRP = 3                # groups per PSUM bank tile (3*160*4B < 2KB)
EPS = 1e-7

_CACHE = {}


def _build_nc(reps=1, skip_routing=False, gp_mode="dve"):
    # reps > 1 wraps the whole computation in an on-device loop; used only by
    # the timing harness (delta of two builds cancels dispatch/transfer cost).
    import contextlib

    import concourse.bacc as bacc
    import concourse.mybir as mybir
    import concourse.tile as tile

    F32 = mybir.dt.float32
    F32R = mybir.dt.float32r
    ALU = mybir.AluOpType
    ACTF = mybir.ActivationFunctionType

    nc = bacc.Bacc()
    # lt: sub-batch 2's block-diagonal stationary tiles (DMA'd under the
    # routing overlap). Sub-batch 1 ships compact (xt) and is expanded
    # on-device (GpSimd mask-multiply) to cut the serial P1 DMA.
    lt_d = nc.dram_tensor("lt", [K2, G2 * 128], F32, kind="ExternalInput")
    xt_d = nc.dram_tensor("xt", [K2, G2 * BB], F32, kind="ExternalInput")
    mlt_d = nc.dram_tensor("mlt", [K2, 128], F32, kind="ExternalInput")
    wr_d = nc.dram_tensor("wr", [K2, FREE2], F32, kind="ExternalInput")
    mask_d = nc.dram_tensor("mask", [128, BB], F32, kind="ExternalInput")
    out_d = nc.dram_tensor("out", [BC, ND], F32, kind="ExternalOutput")

    with tile.TileContext(nc) as tc:
        with (
            tc.tile_pool(name="const", bufs=1) as cpool,
            tc.tile_pool(name="sq", bufs=1) as qpool,
            tc.tile_pool(name="uhp", bufs=NCH) as uhp,
            tc.tile_pool(name="psw", bufs=2, space="PSUM") as swpool,
            tc.tile_pool(name="ltp", bufs=3) as ltp,
            tc.tile_pool(name="wrp", bufs=3) as wrp,
            tc.tile_pool(name="pprod", bufs=6, space="PSUM") as pprod,
            tc.tile_pool(name="x", bufs=3) as xpool,
            tc.tile_pool(name="y", bufs=3) as ypool,
            tc.tile_pool(name="dn", bufs=3) as dnpool,
            tc.tile_pool(name="s1t", bufs=1) as s1pool,
            tc.tile_pool(name="rv", bufs=3) as rvpool,
        ):
            mask_t = cpool.tile([128, BB], F32)
            nc.sync.dma_start(mask_t[:], mask_d[:])
            mlt_t = cpool.tile([K2, 128], F32)
            nc.sync.dma_start(mlt_t[:], mlt_d[:])
            mask_r = cpool.tile([128, BB], F32R)
            nc.vector.tensor_copy(mask_r[:], mask_t[:])

            rep_ctx = (
                tc.For_i(0, reps, 1) if reps > 1 else contextlib.nullcontext()
            )

            def collapse3(ps_w, sc):
                # s_sb = (blk0+blk1+blk2) of [16, 480] PSUM -> [16, 160] SBUF
                cw = qpool.tile([BB, PSUM_GRP * ND], F32, tag="c3_w")
                nc.scalar.activation(cw[:], ps_w[:], ACTF.Copy, scale=sc)
                s3 = qpool.tile([BB, ND], F32, tag="c3_a")
                nc.vector.tensor_add(
                    s3[:], cw[:, 0:ND], cw[:, ND:2 * ND]
                )
                s = qpool.tile([BB, ND], F32, tag="c3_s")
                nc.vector.tensor_add(s[:], s3[:], cw[:, 2 * ND:3 * ND])
                return s

            def bcast16(s):
                # replicate [16, ND] -> [128, ND] (8 partition quadrants)
                s128 = qpool.tile([128, ND], F32, tag="s128")
                for q in range(IPK):
                    nc.sync.dma_start(s128[q * BB:(q + 1) * BB, :], s[:, :])
                return s128

            def squash(s, vt, P=128):
                # vt = squash(s); [P, ND] f32, tiny
                sq = qpool.tile([P, ND], F32, tag="sq_sq")
                nc.vector.tensor_mul(sq[:], s[:], s[:])
                se = qpool.tile([P, ND], F32, tag="sq_se")
                nc.vector.tensor_scalar_add(se[:], sq[:], EPS)
                a = qpool.tile([P, ND], F32, tag="sq_a")
                nc.scalar.activation(a[:], se[:], ACTF.Sqrt)
                d2 = qpool.tile([P, ND], F32, tag="sq_d2")
                nc.vector.scalar_tensor_tensor(
                    d2[:], sq[:], 1.0, a[:], op0=ALU.add, op1=ALU.mult
                )
                r = qpool.tile([P, ND], F32, tag="sq_r")
                r_s = qpool.tile([P, ND], F32, tag="sq_rs")
                nc.vector.reciprocal_approx_accurate(r[:], d2[:], r_s[:])
                t1 = qpool.tile([P, ND], F32, tag="sq_t1")
                nc.vector.tensor_mul(t1[:], s[:], sq[:])
                nc.vector.tensor_mul(vt[:], t1[:], r[:])

            with rep_ctx:
              for s_i in range(SB):
                # V replicated 8x across partitions (p%16 = b): the squash
                # chain directly produces the broadcast tile for the logits.
                # Per-sub tiles so sub 2's routing never waits on sub 1's V.
                V = cpool.tile([128, ND], F32, tag=f"V{s_i}")
                # ---------- production: u_hat + s1 fold ----------
                uch = []
                s1parts = []
                for c in range(NCH):
                    g0 = c * CH
                    ltt = ltp.tile([K2, CH * 128], F32, tag="ltt")
                    if s_i == 0:
                        xtt = ltp.tile([K2, CH * BB], F32, tag="xtt")
                        nc.sync.dma_start(
                            xtt[:], xt_d[:, g0 * BB:(g0 + CH) * BB]
                        )
                        ltt4 = ltt[:].rearrange(
                            "p (g i b) -> p g i b", i=IPK, b=BB
                        )
                        xt_b = (
                            xtt[:]
                            .rearrange("p (g b) -> p g b", b=BB)
                            .unsqueeze(2)
                            .broadcast_to([K2, CH, IPK, BB])
                        )
                        ml_b = (
                            mlt_t[:]
                            .rearrange("p (i b) -> p i b", b=BB)
                            .unsqueeze(1)
                            .broadcast_to([K2, CH, IPK, BB])
                        )
                        nc.gpsimd.tensor_tensor(ltt4, xt_b, ml_b, ALU.mult)
                    else:
                        nc.sync.dma_start(
                            ltt[:],
                            lt_d[:, g0 * 128:(g0 + CH) * 128],
                        )
                    wrt = wrp.tile([K2, CH * ND], F32)
                    nc.sync.dma_start(
                        wrt[:], wr_d[:, g0 * ND:(g0 + CH) * ND]
                    )
                    u = uhp.tile([128, CH * ND], F32, tag="uh")
                    uch.append(u)
                    for t3 in range(CH // PSUM_GRP):
                        pt = pprod.tile([128, PSUM_GRP * ND], F32)
                        for j in range(PSUM_GRP):
                            gl = t3 * PSUM_GRP + j
                            nc.tensor.matmul(
                                pt[:, j * ND:(j + 1) * ND],
                                ltt[:, gl * 128:(gl + 1) * 128],
                                wrt[:, gl * ND:(gl + 1) * ND],
                                start=True,
                                stop=True,
                            )
                        lo = t3 * PSUM_GRP * ND
                        hi = (t3 + 1) * PSUM_GRP * ND
                        nc.scalar.copy(u[:, lo:hi], pt[:])
                    # s1 partial: sum over the chunk's 12 groups on DVE
                    u3c = u[:].rearrange("p (g nd) -> p g nd", nd=ND)
                    t6 = s1pool.tile([128, 6 * ND], F32, tag="s1a")
                    t63 = t6[:].rearrange("p (g nd) -> p g nd", nd=ND)
                    nc.vector.tensor_tensor(
                        t63, u3c[:, 0:12:2, :], u3c[:, 1:12:2, :], ALU.add
                    )
                    t3_ = s1pool.tile([128, 3 * ND], F32, tag="s1b")
                    t33 = t3_[:].rearrange("p (g nd) -> p g nd", nd=ND)
                    nc.vector.tensor_tensor(
                        t33, t63[:, 0:6:2, :], t63[:, 1:6:2, :], ALU.add
                    )
                    sp = s1pool.tile([128, ND], F32, tag="s1p")
                    nc.vector.tensor_add(sp[:], t33[:, 0, :], t33[:, 1, :])
                    nc.vector.tensor_add(sp[:], sp[:], t33[:, 2, :])
                    if c == 0:
                        s1acc = cpool.tile([128, ND], F32, tag="s1acc")
                        nc.vector.tensor_copy(s1acc[:], sp[:])
                    else:
                        nc.vector.tensor_add(s1acc[:], s1acc[:], sp[:])
                # partition fold (i8 quadrants -> b) via one plain-f32 matmul
                ps1 = swpool.tile([BB, ND], F32, tag="psw")
                nc.tensor.matmul(ps1[:], mask_t[:], s1acc[:],
                                 start=True, stop=True)
                s1 = qpool.tile([BB, ND], F32, tag="c3_s")
                nc.scalar.activation(s1[:], ps1[:], ACTF.Copy, scale=0.1)
                squash(bcast16(s1), V)

                # ---------- routing passes 2 and 3 ----------
                for t in () if skip_routing else (2, 3):
                    ps_w = swpool.tile([BB, PSUM_GRP * ND], F32, tag="psw")
                    n_fold = 0
                    for c in range(NCH):
                        u = uch[c]
                        x = xpool.tile([128, CH * ND], F32)
                        x3 = x[:].rearrange("p (g nd) -> p g nd", nd=ND)
                        u3 = u[:].rearrange("p (g nd) -> p g nd", nd=ND)
                        vb_b = V[:].unsqueeze(1).broadcast_to(
                            [128, CH, ND]
                        )
                        # logits = u_hat * V (broadcast over groups)
                        nc.vector.tensor_tensor(x3, u3, vb_b, ALU.mult)
                        nc.scalar.activation(x[:], x[:], ACTF.Exp)
                        # y = e * u_hat, parallel with the GP denom tree;
                        # alternate DVE/GP per chunk for engine balance.
                        # pass 2 folds in exact f32 (its error feeds back
                        # through V and is chaotically amplified); pass 3
                        # folds in f32r (error hits the output directly).
                        y = ypool.tile([128, CH * ND],
                                       F32 if t == 2 else F32R,
                                       tag="y")
                        y_eng = (nc.vector if (c % 2 == 0 or gp_mode == "dve")
                                 else nc.gpsimd)
                        y_eng.tensor_tensor(
                            y[:], x[:], u[:], ALU.mult
                        )
                        # denominator pair-tree over n on GpSimd
                        x4 = x[:].rearrange(
                            "p (g n d) -> p g n d", n=NC, d=DC
                        )
                        # denom = sum over n: one strided reduce on DVE
                        dn = dnpool.tile([128, CH * DC], F32, tag="dn")
                        dn4 = dn[:].rearrange(
                            "p (g o d) -> p g o d", o=1, d=DC
                        )
                        nc.vector.tensor_reduce(
                            dn4,
                            x4.transpose([0, 1, 3, 2]),
                            axis=mybir.AxisListType.X,
                            op=ALU.add,
                        )
                        rv = rvpool.tile([128, CH * DC], F32)
                        rv_s = rvpool.tile([128, CH * DC], F32, tag="rvs")
                        nc.vector.reciprocal_approx_accurate(
                            rv[:], dn[:], rv_s[:]
                        )
                        rv_b = (
                            rv[:]
                            .rearrange("p (g d) -> p g d", d=DC)
                            .unsqueeze(2)
                            .broadcast_to([128, CH, NC, DC])
                        )
                        y4 = y[:].rearrange(
                            "p (g n d) -> p g n d", n=NC, d=DC
                        )
                        nc.vector.tensor_tensor(y4, y4, rv_b, ALU.mult)
                        mk = mask_t if t == 2 else mask_r
                        for j3 in range(CH // PSUM_GRP):
                            nc.tensor.matmul(
                                ps_w[:],
                                mk[:],
                                y[:, j3 * PSUM_GRP * ND:
                                  (j3 + 1) * PSUM_GRP * ND],
                                start=(n_fold == 0),
                                stop=(n_fold == G2 // PSUM_GRP - 1),
                            )
                            n_fold += 1
                    s_t = collapse3(ps_w, 1.0)
                    vt = qpool.tile([128, ND], F32, tag="vt")
                    squash(bcast16(s_t), vt)
                    if t == 2:
                        nc.vector.tensor_add(V[:], V[:], vt[:])
                    else:
                        nc.sync.dma_start(
                            out_d[s_i * BB:(s_i + 1) * BB, :],
                            vt[0:BB, :],
                        )
            if skip_routing:
                for s_i in range(SB):
                    nc.sync.dma_start(
                        out_d[s_i * BB:(s_i + 1) * BB, :], V[0:BB, :]
                    )
    nc.finalize()
    return nc


def _host_pack(inputs, W):
    """Build per-core LT, shared WR and mask, all f32."""
    inputs = np.ascontiguousarray(inputs, dtype=np.float32)
    W = np.ascontiguousarray(W, dtype=np.float32)

    # WR[r=(i8*8+k), g*160 + n*16 + d] = W[n, g*8+i8, d, k]
    W6 = W.reshape(NC, G2, IPK, DC, ID)
    wr = np.ascontiguousarray(
        W6.transpose(2, 4, 1, 0, 3).reshape(K2, FREE2)
    )

    mask = np.ascontiguousarray(
        np.tile(np.eye(BB, dtype=np.float32), (IPK, 1))
    )
    # mlt[r=(i8*8+k), i8'*16+b] = (i8 == i8')
    mlt = np.zeros((K2, 128), dtype=np.float32)
    for i8 in range(IPK):
        mlt[i8 * ID:(i8 + 1) * ID, i8 * BB:(i8 + 1) * BB] = 1.0

    lts, xts = [], []
    for core in range(NCORES):
        xc = inputs[core * BC:(core + 1) * BC]              # [BC, IC, ID]
        x6 = xc.reshape(SB, BB, G2, IPK, ID)                # [s, b, g, i8, k]
        # sub-batch 2: full block-diagonal layout
        lt = np.zeros((K2, G2, 128), dtype=np.float32)
        for i8 in range(IPK):
            lt[i8 * ID:(i8 + 1) * ID, :, i8 * BB:(i8 + 1) * BB] = (
                x6[1, :, :, i8, :].transpose(2, 1, 0)       # [k, g, b]
            )
        lts.append(np.ascontiguousarray(lt.reshape(K2, G2 * 128)))
        # sub-batch 1: compact transposed inputs
        xt = np.zeros((K2, G2, BB), dtype=np.float32)
        for i8 in range(IPK):
            xt[i8 * ID:(i8 + 1) * ID] = x6[0, :, :, i8, :].transpose(2, 1, 0)
        xts.append(np.ascontiguousarray(xt.reshape(K2, G2 * BB)))
    return lts, xts, wr, mask, mlt


def kernel(inputs, W):
    from concourse.bass_utils import run_bass_kernel_spmd

    if "nc" not in _CACHE:
        _CACHE["nc"] = _build_nc()
    nc = _CACHE["nc"]

    lts, xts, wr, mask, mlt = _host_pack(np.asarray(inputs), np.asarray(W))
    in_maps = [
        {"lt": lts[c], "xt": xts[c], "wr": wr, "mask": mask, "mlt": mlt}
        for c in range(NCORES)
    ]
    res = run_bass_kernel_spmd(nc, in_maps, core_ids=list(range(NCORES)))
    outs = [
        np.asarray(res.results[c]["out"]).reshape(BC, NC, DC, 1)
        for c in range(NCORES)
    ]
    return np.concatenate(outs, axis=0).astype(np.float32)


if __name__ == "__main__":
    rng = np.random.default_rng(0)
    x = rng.standard_normal((B, IC, ID), dtype=np.float32)
    w = rng.standard_normal((NC, IC, DC, ID), dtype=np.float32) * 0.1
    out = kernel(x, w)
    print(out.shape, out.dtype)



# revision 22
# speedup vs baseline: 3.3334x; 3.3334x over previous
"""CapsuleLayer (dynamic routing) Trainium2 Bass kernel.

Full-input contract: kernel(inputs, W) -> [256, 10, 16, 1] f32.
Data-parallel over batch: 8 cores x 32 batches, W replicated.

Math restructuring vs the reference:
  - routing logits are always b_t = u_hat * V_t with V_t = sum of previous
    squashed outputs (broadcast over IC), so no [B,NC,IC,DC] logits tensor is
    ever materialized; only the running V[b,n,d] is kept.
  - pass 1 (uniform softmax) reduces to s1 = 0.1 * sum_i u_hat.

u_hat and all pass-2 intermediates are f32: the routing iteration chaotically
amplifies perturbations, so 16-bit intermediates destroy accuracy. Reduced
precision is used only where the error either hits the output directly
(pass-3 capsule folds, f32r) or washes out by CLT over the 1152-capsule fold
(sub-2's s1 fold, f32r), plus the ~18-bit fast reciprocals.

Per core the 32 local batches are processed as 2 serial sub-batches of 16 so
the f32 u_hat stays SBUF-resident:
  partition p = i8*16 + b   (8 input capsules packed per "group", 144 groups)
  u_hat: 12 chunk tiles [128, 12*160] in a 12-slot pool.
  production: u[p, g*160+nd] = sum_{r=(i8,k)} LT[r, g*128+p] * WR[r, g*160+nd]
    with LT the block-diagonal input transposes and WR the matching W slices.
  capsule fold: s[b, nd] = sum_p mask[p, b] * w[p, nd] via PE (mask = eye(16)
    tiled 8x), accumulated 3 groups wide ([16, 480] PSUM).

Schedule (the engines execute in order, so emission order is the schedule):
  production-1 (DMA-bound; s1 pair-tree on the otherwise idle DVE)
  pass-2 of sub-1 (+ prefetch DMAs for sub-2's lt/wr under its DMA slack)
  pass-3 of sub-1 interleaved chunk-by-chunk with production-2 (PE/Act slack;
    s1 via f32r PE mask-folds, where rounding washes out over the 1152-fold)
  pass-2, pass-3 of sub-2
Routing elementwise work is split DVE/GpSimd per the gp_* knobs (chunks
round-robin); softmax denominator reduce + reciprocal stay on DVE (GpSimd
cannot reduce free axes). V broadcasts [16,ND]->[128,ND] ride a PE matmul
with a host-packed replication matrix instead of 8 serial DMAs.
"""

import os
import sys

import numpy as np

sys.path.insert(0, "/opt/trn_rl_repo")

B, IC, ID = 256, 1152, 8
NC, DC = 10, 16
NCORES = 8
BC = B // NCORES            # 32 batches per core
SB = 2                      # sub-batches per core
BB = BC // SB               # 16 batches per sub-batch
IPK = 8                     # input capsules packed per group
G2 = IC // IPK              # 144 groups
K2 = IPK * ID               # 64 contraction rows
ND = NC * DC                # 160
FREE2 = G2 * ND             # 23040
CH = 12                     # chunk size in groups (production & routing)
NCH = G2 // CH              # 12 chunks
PSUM_GRP = 3                # groups per PSUM bank tile (3*160*4B < 2KB)
EPS = 1e-7

_CACHE = {}


def _build_nc(reps=1, skip_routing=False, gp_y=12, gp_norm=12, gp_logits=0):
    # reps > 1 wraps the whole computation in an on-device loop; used only by
    # the timing harness (delta of two builds cancels dispatch/transfer cost).
    import contextlib

    import concourse.bacc as bacc
    import concourse.mybir as mybir
    import concourse.tile as tile

    F32 = mybir.dt.float32
    F32R = mybir.dt.float32r
    ALU = mybir.AluOpType
    ACTF = mybir.ActivationFunctionType

    nc = bacc.Bacc()
    # lt: sub-batch 2's block-diagonal stationary tiles (DMA'd under the
    # routing overlap). Sub-batch 1 ships compact (xt) and is expanded
    # on-device (GpSimd mask-multiply) to cut the serial P1 DMA.
    lt_d = nc.dram_tensor("lt", [K2, G2 * 128], F32, kind="ExternalInput")
    xt_d = nc.dram_tensor("xt", [K2, G2 * BB], F32, kind="ExternalInput")
    mlt_d = nc.dram_tensor("mlt", [K2, 128], F32, kind="ExternalInput")
    wr_d = nc.dram_tensor("wr", [K2, FREE2], F32, kind="ExternalInput")
    mask_d = nc.dram_tensor("mask", [128, BB], F32, kind="ExternalInput")
    bce_d = nc.dram_tensor("bce", [BB, 128], F32, kind="ExternalInput")
    out_d = nc.dram_tensor("out", [BC, ND], F32, kind="ExternalOutput")

    with tile.TileContext(nc) as tc:
        with (
            tc.tile_pool(name="const", bufs=1) as cpool,
            tc.tile_pool(name="sq", bufs=1) as qpool,
            tc.tile_pool(name="uhp", bufs=NCH) as uhp,
            tc.tile_pool(name="psw", bufs=2, space="PSUM") as swpool,
            tc.tile_pool(name="ps1", bufs=1, space="PSUM") as s1psum,
            tc.tile_pool(name="pbc", bufs=1, space="PSUM") as bcpsum,
            tc.tile_pool(name="ltp", bufs=4) as ltp,
            tc.tile_pool(name="xtp", bufs=2) as xtp,
            tc.tile_pool(name="wrp", bufs=4) as wrp,
            tc.tile_pool(name="pprod", bufs=4, space="PSUM") as pprod,
            tc.tile_pool(name="x", bufs=3) as xpool,
            tc.tile_pool(name="y", bufs=2) as ypool,
            tc.tile_pool(name="dn", bufs=3) as dnpool,
            tc.tile_pool(name="s1t", bufs=1) as s1pool,
            tc.tile_pool(name="rv", bufs=2) as rvpool,
        ):
            mask_t = cpool.tile([128, BB], F32)
            nc.sync.dma_start(mask_t[:], mask_d[:])
            mlt_t = cpool.tile([K2, 128], F32)
            nc.sync.dma_start(mlt_t[:], mlt_d[:])
            bce_t = cpool.tile([BB, 128], F32)
            nc.sync.dma_start(bce_t[:], bce_d[:])
            mask_r = cpool.tile([128, BB], F32R)
            nc.vector.tensor_copy(mask_r[:], mask_t[:])

            rep_ctx = (
                tc.For_i(0, reps, 1) if reps > 1 else contextlib.nullcontext()
            )

            def collapse3(ps, sc):
                # s = (blk0+blk1+blk2) of [16, 480] PSUM -> [16, 160] SBUF
                cw = qpool.tile([BB, PSUM_GRP * ND], F32, tag="c3_w")
                nc.scalar.activation(cw[:], ps[:], ACTF.Copy, scale=sc)
                s3 = qpool.tile([BB, ND], F32, tag="c3_a")
                nc.vector.tensor_add(
                    s3[:], cw[:, 0:ND], cw[:, ND:2 * ND]
                )
                s = qpool.tile([BB, ND], F32, tag="c3_s")
                nc.vector.tensor_add(s[:], s3[:], cw[:, 2 * ND:3 * ND])
                return s

            def squash16(s, tag="vt16"):
                # vt = squash(s) on the 16-partition tile; [16, ND] f32
                sq = qpool.tile([BB, ND], F32, tag="sq_sq")
                nc.vector.tensor_mul(sq[:], s[:], s[:])
                se = qpool.tile([BB, ND], F32, tag="sq_se")
                nc.vector.tensor_scalar_add(se[:], sq[:], EPS)
                a = qpool.tile([BB, ND], F32, tag="sq_a")
                nc.scalar.activation(a[:], se[:], ACTF.Sqrt)
                d2 = qpool.tile([BB, ND], F32, tag="sq_d2")
                nc.vector.scalar_tensor_tensor(
                    d2[:], sq[:], 1.0, a[:], op0=ALU.add, op1=ALU.mult
                )
                r = qpool.tile([BB, ND], F32, tag="sq_r")
                nc.vector.reciprocal_approx_fast(r[:], d2[:])
                t1 = qpool.tile([BB, ND], F32, tag="sq_t1")
                nc.vector.tensor_mul(t1[:], s[:], sq[:])
                vt = qpool.tile([BB, ND], F32, tag=tag)
                nc.vector.tensor_mul(vt[:], t1[:], r[:])
                return vt

            def bcast_pe(v16, V):
                # V[128, ND] = v16 replicated into the 8 partition quadrants
                # via one PE matmul (bce = eye(16) tiled) + one Act copy.
                pb = bcpsum.tile([128, ND], F32, tag="bc")
                nc.tensor.matmul(pb[:], bce_t[:], v16[:],
                                 start=True, stop=True)
                nc.scalar.copy(V[:], pb[:])

            def produce_chunk(s_i, c, uch, st, pre=None):
                g0 = c * CH
                if s_i == 0:
                    ltt = ltp.tile([K2, CH * 128], F32, tag="ltt")
                    xtt = xtp.tile([K2, CH * BB], F32, tag="xtt")
                    nc.sync.dma_start(
                        xtt[:], xt_d[:, g0 * BB:(g0 + CH) * BB]
                    )
                    ltt4 = ltt[:].rearrange(
                        "p (g i b) -> p g i b", i=IPK, b=BB
                    )
                    xt_b = (
                        xtt[:]
                        .rearrange("p (g b) -> p g b", b=BB)
                        .unsqueeze(2)
                        .broadcast_to([K2, CH, IPK, BB])
                    )
                    ml_b = (
                        mlt_t[:]
                        .rearrange("p (i b) -> p i b", b=BB)
                        .unsqueeze(1)
                        .broadcast_to([K2, CH, IPK, BB])
                    )
                    nc.gpsimd.tensor_tensor(ltt4, xt_b, ml_b, ALU.mult)
                    wrt = wrp.tile([K2, CH * ND], F32)
                    nc.sync.dma_start(
                        wrt[:], wr_d[:, g0 * ND:(g0 + CH) * ND]
                    )
                else:
                    ltt, wrt = pre[c]
                u = uhp.tile([128, CH * ND], F32, tag="uh")
                uch.append(u)
                for t3 in range(CH // PSUM_GRP):
                    pt = pprod.tile([128, PSUM_GRP * ND], F32)
                    for j in range(PSUM_GRP):
                        gl = t3 * PSUM_GRP + j
                        nc.tensor.matmul(
                            pt[:, j * ND:(j + 1) * ND],
                            ltt[:, gl * 128:(gl + 1) * 128],
                            wrt[:, gl * ND:(gl + 1) * ND],
                            start=True,
                            stop=True,
                        )
                    lo = t3 * PSUM_GRP * ND
                    hi = (t3 + 1) * PSUM_GRP * ND
                    nc.scalar.copy(u[:, lo:hi], pt[:])
                # s1 partials: two pair-tree levels (1920 -> 480 cols), then
                # a cheap f32 PE mask-fold of the quarter-width result.
                # Sub-1's production is DMA/PE-bound with the DVE idle, so
                # both levels ride the DVE; sub-2's production overlaps sub-1
                # routing (DVE saturated), so level 1 goes to GpSimd.
                u3c = u[:].rearrange("p (g nd) -> p g nd", nd=ND)
                t6 = s1pool.tile([128, 6 * ND], F32, tag="s1a")
                t63 = t6[:].rearrange("p (g nd) -> p g nd", nd=ND)
                eng1 = nc.gpsimd if s_i == 1 else nc.vector
                eng1.tensor_tensor(
                    t63, u3c[:, 0:12:2, :], u3c[:, 1:12:2, :], ALU.add
                )
                t3_ = s1pool.tile([128, 3 * ND], F32, tag="s1b")
                t33 = t3_[:].rearrange("p (g nd) -> p g nd", nd=ND)
                nc.vector.tensor_tensor(
                    t33, t63[:, 0:6:2, :], t63[:, 1:6:2, :], ALU.add
                )
                nc.tensor.matmul(
                    st["ps_s1"][:],
                    mask_t[:],
                    t3_[:],
                    start=(c == 0),
                    stop=(c == NCH - 1),
                )

            def s1_finalize(s_i, st, V, v16s):
                s1 = collapse3(st["ps_s1"], 0.1)
                v16 = squash16(s1, tag=f"v16_{s_i}")
                v16s[s_i] = v16
                bcast_pe(v16, V)

            def routing_chunk(s_i, t, c, uch, V, ps_w, st):
                u = uch[c]
                x = xpool.tile([128, CH * ND], F32)
                x3 = x[:].rearrange("p (g nd) -> p g nd", nd=ND)
                u3 = u[:].rearrange("p (g nd) -> p g nd", nd=ND)
                vb_b = V[:].unsqueeze(1).broadcast_to([128, CH, ND])
                # logits = u_hat * V (broadcast over groups)
                lg_eng = (nc.gpsimd if c % 12 < gp_logits else nc.vector)
                lg_eng.tensor_tensor(x3, u3, vb_b, ALU.mult)
                nc.scalar.activation(x[:], x[:], ACTF.Exp)
                # y = e * u_hat; chunks round-robin DVE/GpSimd per the gp_*
                # knobs. pass 2 folds in exact f32 (error feeds back through
                # V, chaotically amplified); pass 3 folds in f32r (error
                # hits the output directly).
                y = ypool.tile([128, CH * ND],
                               F32 if t == 2 else F32R,
                               tag="y")
                y_eng = nc.gpsimd if c % 12 < gp_y else nc.vector
                y_eng.tensor_tensor(y[:], x[:], u[:], ALU.mult)
                x4 = x[:].rearrange("p (g n d) -> p g n d", n=NC, d=DC)
                # denom = sum over n: one strided reduce on DVE
                dn = dnpool.tile([128, CH * DC], F32, tag="dn")
                dn4 = dn[:].rearrange("p (g o d) -> p g o d", o=1, d=DC)
                nc.vector.tensor_reduce(
                    dn4,
                    x4.transpose([0, 1, 3, 2]),
                    axis=mybir.AxisListType.X,
                    op=ALU.add,
                )
                rv = rvpool.tile([128, CH * DC], F32)
                nc.vector.reciprocal_approx_fast(rv[:], dn[:])
                rv_b = (
                    rv[:]
                    .rearrange("p (g d) -> p g d", d=DC)
                    .unsqueeze(2)
                    .broadcast_to([128, CH, NC, DC])
                )
                y4 = y[:].rearrange("p (g n d) -> p g n d", n=NC, d=DC)
                nm_eng = (nc.gpsimd if c % 12 >= 12 - gp_norm else nc.vector)
                nm_eng.tensor_tensor(y4, y4, rv_b, ALU.mult)
                mk = mask_t if t == 2 else mask_r
                for j3 in range(CH // PSUM_GRP):
                    nc.tensor.matmul(
                        ps_w[:],
                        mk[:],
                        y[:, j3 * PSUM_GRP * ND:(j3 + 1) * PSUM_GRP * ND],
                        start=(st["n"] == 0),
                        stop=(st["n"] == G2 // PSUM_GRP - 1),
                    )
                    st["n"] += 1

            def pass_finalize(s_i, t, ps_w, V, v16s):
                s_t = collapse3(ps_w, 1.0)
                vt = squash16(s_t)
                if t == 2:
                    v16 = v16s[s_i]
                    nc.vector.tensor_add(v16[:], v16[:], vt[:])
                    bcast_pe(v16, V)
                else:
                    nc.sync.dma_start(
                        out_d[s_i * BB:(s_i + 1) * BB, :], vt[:, :]
                    )

            with rep_ctx:
                v16s = {}
                Vs = [
                    cpool.tile([128, ND], F32, tag=f"V{si}", name=f"V{si}")
                    for si in range(SB)
                ]
                # ---- production-1 (s1 on DVE) ----
                st1 = {
                    "n": 0,
                    "ps_s1": s1psum.tile(
                        [BB, PSUM_GRP * ND], F32, tag="ps1", name="ps_s1a"
                    ),
                }
                uch1 = []
                for c in range(NCH):
                    produce_chunk(0, c, uch1, st1)
                s1_finalize(0, st1, Vs[0], v16s)

                if skip_routing:
                    for s_i in range(SB):
                        nc.sync.dma_start(
                            out_d[s_i * BB:(s_i + 1) * BB, :],
                            Vs[0][0:BB, :],
                        )
                else:
                    # ---- pass 2 of sub-1, with sub-2 lt/wr prefetch ----
                    pre = []
                    rt = {"n": 0}
                    ps_w = swpool.tile([BB, PSUM_GRP * ND], F32, tag="psw")
                    for c in range(NCH):
                        routing_chunk(0, 2, c, uch1, Vs[0], ps_w, rt)
                        g0 = c * CH
                        ltt = ltp.tile([K2, CH * 128], F32, tag="ltt")
                        nc.sync.dma_start(
                            ltt[:], lt_d[:, g0 * 128:(g0 + CH) * 128]
                        )
                        wrt = wrp.tile([K2, CH * ND], F32)
                        nc.sync.dma_start(
                            wrt[:], wr_d[:, g0 * ND:(g0 + CH) * ND]
                        )
                        pre.append((ltt, wrt))
                    pass_finalize(0, 2, ps_w, Vs[0], v16s)

                    # ---- pass 3 of sub-1 interleaved with production-2 ----
                    st2 = {
                        "n": 0,
                        "ps_s1": s1psum.tile(
                            [BB, PSUM_GRP * ND], F32, tag="ps1",
                            name="ps_s1",
                        ),
                    }
                    uch2 = []
                    rt = {"n": 0}
                    ps_w = swpool.tile([BB, PSUM_GRP * ND], F32, tag="psw")
                    for c in range(NCH):
                        routing_chunk(0, 3, c, uch1, Vs[0], ps_w, rt)
                        produce_chunk(1, c, uch2, st2, pre=pre)
                    pass_finalize(0, 3, ps_w, Vs[0], v16s)
                    s1_finalize(1, st2, Vs[1], v16s)

                    # ---- pass 2 and 3 of sub-2 ----
                    for t in (2, 3):
                        rt = {"n": 0}
                        ps_w = swpool.tile(
                            [BB, PSUM_GRP * ND], F32, tag="psw"
                        )
                        for c in range(NCH):
                            routing_chunk(1, t, c, uch2, Vs[1], ps_w, rt)
                        pass_finalize(1, t, ps_w, Vs[1], v16s)
    nc.finalize()
    return nc


def _host_pack(inputs, W):
    """Build per-core LT, shared WR and mask, all f32."""
    inputs = np.ascontiguousarray(inputs, dtype=np.float32)
    W = np.ascontiguousarray(W, dtype=np.float32)

    # WR[r=(i8*8+k), g*160 + n*16 + d] = W[n, g*8+i8, d, k]
    W6 = W.reshape(NC, G2, IPK, DC, ID)
    wr = np.ascontiguousarray(
        W6.transpose(2, 4, 1, 0, 3).reshape(K2, FREE2)
    )

    mask = np.ascontiguousarray(
        np.tile(np.eye(BB, dtype=np.float32), (IPK, 1))
    )
    bce = np.ascontiguousarray(mask.T)
    # mlt[r=(i8*8+k), i8'*16+b] = (i8 == i8')
    mlt = np.zeros((K2, 128), dtype=np.float32)
    for i8 in range(IPK):
        mlt[i8 * ID:(i8 + 1) * ID, i8 * BB:(i8 + 1) * BB] = 1.0

    lts, xts = [], []
    for core in range(NCORES):
        xc = inputs[core * BC:(core + 1) * BC]              # [BC, IC, ID]
        x6 = xc.reshape(SB, BB, G2, IPK, ID)                # [s, b, g, i8, k]
        # sub-batch 2: full block-diagonal layout
        lt = np.zeros((K2, G2, 128), dtype=np.float32)
        for i8 in range(IPK):
            lt[i8 * ID:(i8 + 1) * ID, :, i8 * BB:(i8 + 1) * BB] = (
                x6[1, :, :, i8, :].transpose(2, 1, 0)       # [k, g, b]
            )
        lts.append(np.ascontiguousarray(lt.reshape(K2, G2 * 128)))
        # sub-batch 1: compact transposed inputs
        xt = np.zeros((K2, G2, BB), dtype=np.float32)
        for i8 in range(IPK):
            xt[i8 * ID:(i8 + 1) * ID] = x6[0, :, :, i8, :].transpose(2, 1, 0)
        xts.append(np.ascontiguousarray(xt.reshape(K2, G2 * BB)))
    return lts, xts, wr, mask, mlt, bce


def kernel(inputs, W):
    from concourse.bass_utils import run_bass_kernel_spmd

    if "nc" not in _CACHE:
        _CACHE["nc"] = _build_nc()
    nc = _CACHE["nc"]

    lts, xts, wr, mask, mlt, bce = _host_pack(
        np.asarray(inputs), np.asarray(W)
    )
    in_maps = [
        {"lt": lts[c], "xt": xts[c], "wr": wr, "mask": mask, "mlt": mlt,
         "bce": bce}
        for c in range(NCORES)
    ]
    res = run_bass_kernel_spmd(nc, in_maps, core_ids=list(range(NCORES)))
    outs = [
        np.asarray(res.results[c]["out"]).reshape(BC, NC, DC, 1)
        for c in range(NCORES)
    ]
    return np.concatenate(outs, axis=0).astype(np.float32)


if __name__ == "__main__":
    rng = np.random.default_rng(0)
    x = rng.standard_normal((B, IC, ID), dtype=np.float32)
    w = rng.standard_normal((NC, IC, DC, ID), dtype=np.float32) * 0.1
    out = kernel(x, w)
    print(out.shape, out.dtype)


# revision 34
# speedup vs baseline: 3.3827x; 1.0148x over previous
"""CapsuleLayer (dynamic routing) Trainium2 Bass kernel.

Full-input contract: kernel(inputs, W) -> [256, 10, 16, 1] f32.
Data-parallel over batch: 8 cores x 32 batches, W replicated.

Math restructuring vs the reference:
  - routing logits are always b_t = u_hat * V_t with V_t = sum of previous
    squashed outputs (broadcast over IC), so no [B,NC,IC,DC] logits tensor is
    ever materialized; only the running V[b,n,d] is kept.
  - pass 1 (uniform softmax) reduces to s1 = 0.1 * sum_i u_hat.

u_hat and all pass-2 intermediates are f32: the routing iteration chaotically
amplifies perturbations, so 16-bit intermediates destroy accuracy. Reduced
precision is used only where the error either hits the output directly
(pass-3 capsule folds, f32r) or washes out by CLT over the 1152-capsule fold
(sub-2's s1 fold, f32r), plus the ~18-bit fast reciprocals.

Per core the 32 local batches are processed as 2 serial sub-batches of 16 so
the f32 u_hat stays SBUF-resident:
  partition p = i8*16 + b   (8 input capsules packed per "group", 144 groups)
  u_hat: 12 chunk tiles [128, 12*160] in a 12-slot pool.
  production: u[p, g*160+nd] = sum_{r=(i8,k)} LT[r, g*128+p] * WR[r, g*160+nd]
    with LT the block-diagonal input transposes and WR the matching W slices.
  capsule fold: s[b, nd] = sum_p mask[p, b] * w[p, nd] via PE (mask = eye(16)
    tiled 8x), accumulated 3 groups wide ([16, 480] PSUM).

Schedule (the engines execute in order, so emission order is the schedule):
  production-1 (DMA-bound; s1 pair-tree levels on the otherwise idle DVE,
    then a cheap f32 PE mask-fold of the quarter-width partials)
  pass-2 of sub-1 (+ prefetch DMAs for sub-2's lt/wr under its DMA slack)
  pass-3 of sub-1 interleaved chunk-by-chunk with production-2 (PE/Act slack)
  pass-2, pass-3 of sub-2
All elementwise work stays on the DVE: same-process A/Bs on HW showed every
GpSimd offload variant to be net-negative (GpSimd TT runs at ~0.4x DVE rate
per the Q7 software loops, and concurrent Pool+DVE streaming contend for the
shared SBUF ports), hence the gp_* knobs default to 0. u_hat/exp/y are kept
in (g, d, n) element order so the softmax-denominator reduce is a contiguous
inner-axis reduce and the normalize broadcasts 1/D with an inner stride-0 AP
(both measurably faster than the strided/outer-broadcast forms). V
broadcasts [16,ND]->[128,ND] ride a PE matmul with a host-packed replication
matrix instead of 8 serial DMAs.
"""

import os
import sys

import numpy as np

sys.path.insert(0, "/opt/trn_rl_repo")

B, IC, ID = 256, 1152, 8
NC, DC = 10, 16
NCORES = 8
BC = B // NCORES            # 32 batches per core
SB = 2                      # sub-batches per core
BB = BC // SB               # 16 batches per sub-batch
IPK = 8                     # input capsules packed per group
G2 = IC // IPK              # 144 groups
K2 = IPK * ID               # 64 contraction rows
ND = NC * DC                # 160
FREE2 = G2 * ND             # 23040
CH = 12                     # chunk size in groups (production & routing)
NCH = G2 // CH              # 12 chunks
PSUM_GRP = 3                # groups per PSUM bank tile (3*160*4B < 2KB)
EPS = 1e-7

_CACHE = {}


def _build_nc(reps=1, skip_routing=False, gp_y=0, gp_norm=0, gp_logits=0,
              tail_early=True, s1_gp=False, xt_gp=False):
    # reps > 1 wraps the whole computation in an on-device loop; used only by
    # the timing harness (delta of two builds cancels dispatch/transfer cost).
    import contextlib

    import concourse.bacc as bacc
    import concourse.mybir as mybir
    import concourse.tile as tile

    F32 = mybir.dt.float32
    F32R = mybir.dt.float32r
    ALU = mybir.AluOpType
    ACTF = mybir.ActivationFunctionType

    nc = bacc.Bacc()
    # lt: sub-batch 2's block-diagonal stationary tiles (DMA'd under the
    # routing overlap). Sub-batch 1 ships compact (xt) and is expanded
    # on-device (GpSimd mask-multiply) to cut the serial P1 DMA.
    lt_d = nc.dram_tensor("lt", [K2, G2 * 128], F32, kind="ExternalInput")
    xt_d = nc.dram_tensor("xt", [K2, G2 * BB], F32, kind="ExternalInput")
    mlt_d = nc.dram_tensor("mlt", [K2, 128], F32, kind="ExternalInput")
    wr_d = nc.dram_tensor("wr", [K2, FREE2], F32, kind="ExternalInput")
    mask_d = nc.dram_tensor("mask", [128, BB], F32, kind="ExternalInput")
    bce_d = nc.dram_tensor("bce", [BB, 128], F32, kind="ExternalInput")
    out_d = nc.dram_tensor("out", [BC, ND], F32, kind="ExternalOutput")

    with tile.TileContext(nc) as tc:
        with (
            tc.tile_pool(name="const", bufs=1) as cpool,
            tc.tile_pool(name="sq", bufs=1) as qpool,
            tc.tile_pool(name="uhp", bufs=NCH) as uhp,
            tc.tile_pool(name="psw", bufs=2, space="PSUM") as swpool,
            tc.tile_pool(name="ps1", bufs=1, space="PSUM") as s1psum,
            tc.tile_pool(name="pbc", bufs=1, space="PSUM") as bcpsum,
            tc.tile_pool(name="ltp", bufs=4) as ltp,
            tc.tile_pool(name="xtp", bufs=2) as xtp,
            tc.tile_pool(name="wrp", bufs=4) as wrp,
            tc.tile_pool(name="pprod", bufs=4, space="PSUM") as pprod,
            tc.tile_pool(name="x", bufs=3) as xpool,
            tc.tile_pool(name="y", bufs=2) as ypool,
            tc.tile_pool(name="dn", bufs=3) as dnpool,
            tc.tile_pool(name="s1t", bufs=1) as s1pool,
            tc.tile_pool(name="rv", bufs=2) as rvpool,
        ):
            mask_t = cpool.tile([128, BB], F32)
            nc.sync.dma_start(mask_t[:], mask_d[:])
            mlt_t = cpool.tile([K2, 128], F32)
            nc.sync.dma_start(mlt_t[:], mlt_d[:])
            bce_t = cpool.tile([BB, 128], F32)
            nc.sync.dma_start(bce_t[:], bce_d[:])
            mask_r = cpool.tile([128, BB], F32R)
            nc.vector.tensor_copy(mask_r[:], mask_t[:])

            rep_ctx = (
                tc.For_i(0, reps, 1) if reps > 1 else contextlib.nullcontext()
            )

            def collapse3(ps, sc):
                # s = (blk0+blk1+blk2) of [16, 480] PSUM -> [16, 160] SBUF
                cw = qpool.tile([BB, PSUM_GRP * ND], F32, tag="c3_w")
                nc.scalar.activation(cw[:], ps[:], ACTF.Copy, scale=sc)
                s3 = qpool.tile([BB, ND], F32, tag="c3_a")
                nc.vector.tensor_add(
                    s3[:], cw[:, 0:ND], cw[:, ND:2 * ND]
                )
                s = qpool.tile([BB, ND], F32, tag="c3_s")
                nc.vector.tensor_add(s[:], s3[:], cw[:, 2 * ND:3 * ND])
                return s

            def squash16(s, tag="vt16"):
                # vt = squash(s) on the 16-partition tile; [16, ND] f32
                sq = qpool.tile([BB, ND], F32, tag="sq_sq")
                nc.vector.tensor_mul(sq[:], s[:], s[:])
                se = qpool.tile([BB, ND], F32, tag="sq_se")
                nc.vector.tensor_scalar_add(se[:], sq[:], EPS)
                a = qpool.tile([BB, ND], F32, tag="sq_a")
                nc.scalar.activation(a[:], se[:], ACTF.Sqrt)
                d2 = qpool.tile([BB, ND], F32, tag="sq_d2")
                nc.vector.scalar_tensor_tensor(
                    d2[:], sq[:], 1.0, a[:], op0=ALU.add, op1=ALU.mult
                )
                r = qpool.tile([BB, ND], F32, tag="sq_r")
                nc.vector.reciprocal_approx_fast(r[:], d2[:])
                t1 = qpool.tile([BB, ND], F32, tag="sq_t1")
                nc.vector.tensor_mul(t1[:], s[:], sq[:])
                vt = qpool.tile([BB, ND], F32, tag=tag)
                nc.vector.tensor_mul(vt[:], t1[:], r[:])
                return vt

            def bcast_pe(v16, V):
                # V[128, ND] = v16 replicated into the 8 partition quadrants
                # via one PE matmul (bce = eye(16) tiled) + one Act copy.
                pb = bcpsum.tile([128, ND], F32, tag="bc")
                nc.tensor.matmul(pb[:], bce_t[:], v16[:],
                                 start=True, stop=True)
                nc.scalar.copy(V[:], pb[:])

            def produce_chunk(s_i, c, uch, st, pre=None):
                g0 = c * CH
                if s_i == 0:
                    ltt = ltp.tile([K2, CH * 128], F32, tag="ltt")
                    xtt = xtp.tile([K2, CH * BB], F32, tag="xtt")
                    nc.sync.dma_start(
                        xtt[:], xt_d[:, g0 * BB:(g0 + CH) * BB]
                    )
                    ltt4 = ltt[:].rearrange(
                        "p (g i b) -> p g i b", i=IPK, b=BB
                    )
                    xt_b = (
                        xtt[:]
                        .rearrange("p (g b) -> p g b", b=BB)
                        .unsqueeze(2)
                        .broadcast_to([K2, CH, IPK, BB])
                    )
                    ml_b = (
                        mlt_t[:]
                        .rearrange("p (i b) -> p i b", b=BB)
                        .unsqueeze(1)
                        .broadcast_to([K2, CH, IPK, BB])
                    )
                    xe = nc.gpsimd if xt_gp else nc.vector
                    xe.tensor_tensor(ltt4, xt_b, ml_b, ALU.mult)
                    wrt = wrp.tile([K2, CH * ND], F32)
                    nc.sync.dma_start(
                        wrt[:], wr_d[:, g0 * ND:(g0 + CH) * ND]
                    )
                else:
                    ltt, wrt = pre[c]
                u = uhp.tile([128, CH * ND], F32, tag="uh")
                uch.append(u)
                for t3 in range(CH // PSUM_GRP):
                    pt = pprod.tile([128, PSUM_GRP * ND], F32)
                    for j in range(PSUM_GRP):
                        gl = t3 * PSUM_GRP + j
                        nc.tensor.matmul(
                            pt[:, j * ND:(j + 1) * ND],
                            ltt[:, gl * 128:(gl + 1) * 128],
                            wrt[:, gl * ND:(gl + 1) * ND],
                            start=True,
                            stop=True,
                        )
                    lo = t3 * PSUM_GRP * ND
                    hi = (t3 + 1) * PSUM_GRP * ND
                    nc.scalar.copy(u[:, lo:hi], pt[:])
                # s1 partials: two pair-tree levels (1920 -> 480 cols), then
                # a cheap f32 PE mask-fold of the quarter-width result.
                # Sub-1's production is DMA/PE-bound with the DVE idle, so
                # both levels ride the DVE; sub-2's production overlaps sub-1
                # routing (DVE saturated), so level 1 goes to GpSimd.
                u3c = u[:].rearrange("p (g nd) -> p g nd", nd=ND)
                t6 = s1pool.tile([128, 6 * ND], F32, tag="s1a")
                t63 = t6[:].rearrange("p (g nd) -> p g nd", nd=ND)
                eng1 = nc.gpsimd if (s_i == 1 and s1_gp) else nc.vector
                eng1.tensor_tensor(
                    t63, u3c[:, 0:12:2, :], u3c[:, 1:12:2, :], ALU.add
                )
                t3_ = s1pool.tile([128, 3 * ND], F32, tag="s1b")
                t33 = t3_[:].rearrange("p (g nd) -> p g nd", nd=ND)
                nc.vector.tensor_tensor(
                    t33, t63[:, 0:6:2, :], t63[:, 1:6:2, :], ALU.add
                )
                nc.tensor.matmul(
                    st["ps_s1"][:],
                    mask_t[:],
                    t3_[:],
                    start=(c == 0),
                    stop=(c == NCH - 1),
                )

            def s1_finalize(s_i, st, V, v16s):
                s1 = collapse3(st["ps_s1"], 0.1)
                v16 = squash16(s1, tag=f"v16_{s_i}")
                v16s[s_i] = v16
                bcast_pe(v16, V)

            def routing_head(s_i, t, c, uch, V, tail):
                u = uch[c]
                x = xpool.tile([128, CH * ND], F32)
                x3 = x[:].rearrange("p (g nd) -> p g nd", nd=ND)
                u3 = u[:].rearrange("p (g nd) -> p g nd", nd=ND)
                vb_b = V[:].unsqueeze(1).broadcast_to([128, CH, ND])
                # logits = u_hat * V (broadcast over groups)
                lg_eng = (nc.gpsimd if c % 12 < gp_logits else nc.vector)
                lg_eng.tensor_tensor(x3, u3, vb_b, ALU.mult)
                nc.scalar.activation(x[:], x[:], ACTF.Exp)
                # previous chunk's normalize+folds emitted here: they fill
                # the in-order DVE queue while y below waits on the Act exp.
                if tail_early and tail is not None:
                    tail()
                # y = e * u_hat. pass 2 folds in exact f32 (error feeds back
                # through V, chaotically amplified); pass 3 folds in f32r
                # (error hits the output directly).
                y = ypool.tile([128, CH * ND],
                               F32 if t == 2 else F32R,
                               tag="y")
                y_eng = nc.gpsimd if c % 12 < gp_y else nc.vector
                y_eng.tensor_tensor(y[:], x[:], u[:], ALU.mult)
                x4 = x[:].rearrange("p (g d n) -> p g d n", n=NC, d=DC)
                # denom = sum over n: contiguous inner reduce on DVE
                dn = dnpool.tile([128, CH * DC], F32, tag="dn")
                dn4 = dn[:].rearrange("p (g d o) -> p g d o", o=1, d=DC)
                nc.vector.tensor_reduce(
                    dn4, x4, axis=mybir.AxisListType.X, op=ALU.add,
                )
                rv = rvpool.tile([128, CH * DC], F32)
                nc.vector.reciprocal_approx_fast(rv[:], dn[:])
                return y, rv

            def routing_tail(t, c, y, rv, ps_w, st):
                # one chunk behind routing_chunk: by the time the normalize
                # hits the in-order GpSimd queue its reciprocal is ready, so
                # Pool never stalls with later chunks' work queued behind it.
                rv_b = (
                    rv[:]
                    .rearrange("p (g d) -> p g d", d=DC)
                    .unsqueeze(3)
                    .broadcast_to([128, CH, DC, NC])
                )
                y4 = y[:].rearrange("p (g d n) -> p g d n", n=NC, d=DC)
                nm_eng = (nc.gpsimd if c % 12 >= 12 - gp_norm else nc.vector)
                nm_eng.tensor_tensor(y4, y4, rv_b, ALU.mult)
                mk = mask_t if t == 2 else mask_r
                for j3 in range(CH // PSUM_GRP):
                    nc.tensor.matmul(
                        ps_w[:],
                        mk[:],
                        y[:, j3 * PSUM_GRP * ND:(j3 + 1) * PSUM_GRP * ND],
                        start=(st["n"] == 0),
                        stop=(st["n"] == G2 // PSUM_GRP - 1),
                    )
                    st["n"] += 1

            def emit_pass(s_i, t, uch, V, ps_w, rt, extra=None):
                pend = None
                for c in range(NCH):
                    tail = (
                        None if pend is None
                        else (lambda p=pend: routing_tail(
                            t, p[0], p[1], p[2], ps_w, rt))
                    )
                    pend = (c,) + routing_head(s_i, t, c, uch, V, tail)
                    if not tail_early and tail is not None:
                        tail()
                    if extra is not None:
                        extra(c)
                routing_tail(t, pend[0], pend[1], pend[2], ps_w, rt)

            def pass_finalize(s_i, t, ps_w, V, v16s):
                s_t = collapse3(ps_w, 1.0)
                vt = squash16(s_t)
                if t == 2:
                    v16 = v16s[s_i]
                    nc.vector.tensor_add(v16[:], v16[:], vt[:])
                    bcast_pe(v16, V)
                else:
                    vo = qpool.tile([BB, ND], F32, tag="vo")
                    vo3 = vo[:].rearrange("p (n d) -> p n d", d=DC)
                    vt3 = (
                        vt[:]
                        .rearrange("p (d n) -> p d n", n=NC)
                        .transpose([0, 2, 1])
                    )
                    nc.vector.tensor_copy(vo3, vt3)
                    nc.sync.dma_start(
                        out_d[s_i * BB:(s_i + 1) * BB, :], vo[:, :]
                    )

            with rep_ctx:
                v16s = {}
                Vs = [
                    cpool.tile([128, ND], F32, tag=f"V{si}", name=f"V{si}")
                    for si in range(SB)
                ]
                # ---- production-1 (s1 on DVE) ----
                st1 = {
                    "n": 0,
                    "ps_s1": s1psum.tile(
                        [BB, PSUM_GRP * ND], F32, tag="ps1", name="ps_s1a"
                    ),
                }
                uch1 = []
                for c in range(NCH):
                    produce_chunk(0, c, uch1, st1)
                s1_finalize(0, st1, Vs[0], v16s)

                if skip_routing:
                    for s_i in range(SB):
                        nc.sync.dma_start(
                            out_d[s_i * BB:(s_i + 1) * BB, :],
                            Vs[0][0:BB, :],
                        )
                else:
                    # ---- pass 2 of sub-1, with sub-2 lt/wr prefetch ----
                    pre = []

                    def prefetch(c):
                        g0 = c * CH
                        ltt = ltp.tile([K2, CH * 128], F32, tag="ltt")
                        nc.sync.dma_start(
                            ltt[:], lt_d[:, g0 * 128:(g0 + CH) * 128]
                        )
                        wrt = wrp.tile([K2, CH * ND], F32, name="wrt")
                        nc.sync.dma_start(
                            wrt[:], wr_d[:, g0 * ND:(g0 + CH) * ND]
                        )
                        pre.append((ltt, wrt))

                    rt = {"n": 0}
                    ps_w = swpool.tile([BB, PSUM_GRP * ND], F32, tag="psw")
                    emit_pass(0, 2, uch1, Vs[0], ps_w, rt, extra=prefetch)
                    pass_finalize(0, 2, ps_w, Vs[0], v16s)

                    # ---- pass 3 of sub-1 interleaved with production-2 ----
                    st2 = {
                        "n": 0,
                        "ps_s1": s1psum.tile(
                            [BB, PSUM_GRP * ND], F32, tag="ps1",
                            name="ps_s1",
                        ),
                    }
                    uch2 = []
                    rt = {"n": 0}
                    ps_w = swpool.tile([BB, PSUM_GRP * ND], F32, tag="psw")
                    emit_pass(
                        0, 3, uch1, Vs[0], ps_w, rt,
                        extra=lambda c: produce_chunk(1, c, uch2, st2,
                                                      pre=pre),
                    )
                    pass_finalize(0, 3, ps_w, Vs[0], v16s)
                    s1_finalize(1, st2, Vs[1], v16s)

                    # ---- pass 2 and 3 of sub-2 ----
                    for t in (2, 3):
                        rt = {"n": 0}
                        ps_w = swpool.tile(
                            [BB, PSUM_GRP * ND], F32, tag="psw"
                        )
                        emit_pass(1, t, uch2, Vs[1], ps_w, rt)
                        pass_finalize(1, t, ps_w, Vs[1], v16s)
    nc.finalize()
    return nc


def _host_pack(inputs, W):
    """Build per-core LT, shared WR and mask, all f32."""
    inputs = np.ascontiguousarray(inputs, dtype=np.float32)
    W = np.ascontiguousarray(W, dtype=np.float32)

    # WR[r=(i8*8+k), g*160 + d*10 + n] = W[n, g*8+i8, d, k]
    # ((g,d,n) element order: contiguous softmax-denominator reduce over n
    # and inner-stride-0 broadcast for the normalize)
    W6 = W.reshape(NC, G2, IPK, DC, ID)
    wr = np.ascontiguousarray(
        W6.transpose(2, 4, 1, 3, 0).reshape(K2, FREE2)
    )

    mask = np.ascontiguousarray(
        np.tile(np.eye(BB, dtype=np.float32), (IPK, 1))
    )
    bce = np.ascontiguousarray(mask.T)
    # mlt[r=(i8*8+k), i8'*16+b] = (i8 == i8')
    mlt = np.zeros((K2, 128), dtype=np.float32)
    for i8 in range(IPK):
        mlt[i8 * ID:(i8 + 1) * ID, i8 * BB:(i8 + 1) * BB] = 1.0

    lts, xts = [], []
    for core in range(NCORES):
        xc = inputs[core * BC:(core + 1) * BC]              # [BC, IC, ID]
        x6 = xc.reshape(SB, BB, G2, IPK, ID)                # [s, b, g, i8, k]
        # sub-batch 2: full block-diagonal layout
        lt = np.zeros((K2, G2, 128), dtype=np.float32)
        for i8 in range(IPK):
            lt[i8 * ID:(i8 + 1) * ID, :, i8 * BB:(i8 + 1) * BB] = (
                x6[1, :, :, i8, :].transpose(2, 1, 0)       # [k, g, b]
            )
        lts.append(np.ascontiguousarray(lt.reshape(K2, G2 * 128)))
        # sub-batch 1: compact transposed inputs
        xt = np.zeros((K2, G2, BB), dtype=np.float32)
        for i8 in range(IPK):
            xt[i8 * ID:(i8 + 1) * ID] = x6[0, :, :, i8, :].transpose(2, 1, 0)
        xts.append(np.ascontiguousarray(xt.reshape(K2, G2 * BB)))
    return lts, xts, wr, mask, mlt, bce


def kernel(inputs, W):
    from concourse.bass_utils import run_bass_kernel_spmd

    if "nc" not in _CACHE:
        _CACHE["nc"] = _build_nc()
    nc = _CACHE["nc"]

    lts, xts, wr, mask, mlt, bce = _host_pack(
        np.asarray(inputs), np.asarray(W)
    )
    in_maps = [
        {"lt": lts[c], "xt": xts[c], "wr": wr, "mask": mask, "mlt": mlt,
         "bce": bce}
        for c in range(NCORES)
    ]
    res = run_bass_kernel_spmd(nc, in_maps, core_ids=list(range(NCORES)))
    outs = [
        np.asarray(res.results[c]["out"]).reshape(BC, NC, DC, 1)
        for c in range(NCORES)
    ]
    return np.concatenate(outs, axis=0).astype(np.float32)


if __name__ == "__main__":
    rng = np.random.default_rng(0)
    x = rng.standard_normal((B, IC, ID), dtype=np.float32)
    w = rng.standard_normal((NC, IC, DC, ID), dtype=np.float32) * 0.1
    out = kernel(x, w)
    print(out.shape, out.dtype)
